# revision 6
# baseline (speedup 1.0000x reference)
"""Trainium2 Bass kernel for GNN message passing.

Computation (see reference):
    H   = X @ W + b
    out = dropout(relu(segment_sum(H[cols] * vals, rows)), p=0.1, key=42)

Algebraic restructure used here (bias commutes through the sparse matmul):
    S[d]    = sum_{e: rows[e]==d} vals[e] * X[cols[e]]        (feature gather + segment sum)
    degw[d] = sum_{e: rows[e]==d} vals[e]                      (host-side bincount)
    out[d]  = relu(S[d] @ W + degw[d] * b) * mask[d]           (mask = dropout keep/0.9)

Sharding: destination nodes (rows of the output) are sharded across the 8
NeuronCores; each core owns 12500 contiguous destination rows and all edges
pointing at them. Edges are sorted by (window-group, source-bank, window)
where a "window" is 512 destination rows. Per 128-edge chunk the kernel:
  1. dma_gather's the 128 source rows (fp16, 256B each) from HBM,
  2. builds a one-hot [128 edges x 512 dests] matrix P carrying vals via a
     single tensor_scalar (iota == off) * val,
  3. matmul-accumulates S^T[feature, dest] += msgs^T @ P into PSUM.
Then per window: S@W via PE, rank-1 degw*b bias matmul, fused relu+mask on
DVE, and a dense store of the output rows.

The Bass program is SPMD (one NEFF for all 8 cores): static chunk counts per
(window, bank) are the max over cores, with unused slots padded (val=0).
"""

import sys

for _p in ("/opt/trn_rl_repo", "/root/.axon_site/_ro/trn_rl_repo"):
    if _p not in sys.path:
        sys.path.insert(0, _p)

import numpy as np

DEFAULT_CFG = dict(
    N=100000,      # nodes
    F=128,         # features (in = out = 128)
    NC=8,          # cores
    WIN=512,       # destination window width (one PSUM bank of fp32)
    GRP=5,         # windows per group (PSUM residency)
    BANK=25000,    # gather-table bank rows (int16 index limit)
    DROP_P=0.1,
)

_nc_cache = {}


def _plan(rows, cols, vals, cfg):
    """Host-side edge layout. Returns the shared static structure + per-core arrays."""
    N, NC, WIN, GRP, BANK = cfg["N"], cfg["NC"], cfg["WIN"], cfg["GRP"], cfg["BANK"]
    n_core = N // NC
    nwin = -(-n_core // WIN)
    ngrp = -(-nwin // GRP)
    nbank = -(-N // BANK)

    rows = np.asarray(rows).astype(np.int64)
    cols = np.asarray(cols).astype(np.int64)
    vals = np.asarray(vals).astype(np.float32)

    core = rows // n_core
    rl = rows % n_core
    w = rl // WIN
    g = w // GRP
    b = cols // BANK
    off = (rl % WIN).astype(np.float32)
    idxb = (cols % BANK).astype(np.int16)

    # per (core, w, b) counts
    seg_of_edge = (w * nbank + b).astype(np.int64)
    nseg = nwin * nbank
    counts = np.zeros((NC, nseg), np.int64)
    for c in range(NC):
        m = core == c
        counts[c] = np.bincount(seg_of_edge[m], minlength=nseg)
    cmax = counts.max(axis=0)
    C_wb = -(-cmax // 128)  # chunks per (w,b), shared across cores; may be 0
    C_wb = C_wb.reshape(nwin, nbank)

    # stream layout: for g, for b, for w in group g -> C_wb[w][b] chunks
    seg_start_chunk = np.zeros((nwin, nbank), np.int64)
    stream = []  # list of (g, b, w, chunk_count, start_chunk)
    pos = 0
    for gi in range(ngrp):
        for bi in range(nbank):
            for wi in range(gi * GRP, min((gi + 1) * GRP, nwin)):
                seg_start_chunk[wi, bi] = pos
                stream.append((gi, bi, wi, int(C_wb[wi, bi]), pos))
                pos += int(C_wb[wi, bi])
    total_chunks = pos
    total_slots = total_chunks * 128

    # gather splits: per (g,b) contiguous chunk-range split into <=8-chunk calls
    gathers = []  # (bank, start_chunk, n_chunks)
    for gi in range(ngrp):
        for bi in range(nbank):
            lo = None
            hi = None
            for (g2, b2, w2, nch, st) in stream:
                if g2 == gi and b2 == bi and nch > 0:
                    if lo is None:
                        lo = st
                    hi = st + nch
            if lo is None:
                continue
            c0 = lo
            while c0 < hi:
                nch = min(8, hi - c0)
                gathers.append((bi, c0, nch))
                c0 += nch

    # per-core slot arrays
    idx_slots = np.zeros((NC, total_slots), np.int16)
    off_slots = np.zeros((NC, total_slots), np.float32)
    val_slots = np.zeros((NC, total_slots), np.float32)
    seg_start_slot = seg_start_chunk * 128
    for c in range(NC):
        m = core == c
        segs = seg_of_edge[m]
        order = np.argsort(segs, kind="stable")
        segs_o = segs[order]
        # rank within segment
        first = np.r_[True, segs_o[1:] != segs_o[:-1]]
        idx_first = np.flatnonzero(first)
        seg_sizes = np.diff(np.r_[idx_first, len(segs_o)])
        rank = np.arange(len(segs_o)) - np.repeat(idx_first, seg_sizes)
        wv = segs_o // nbank
        bv = segs_o % nbank
        slot = seg_start_slot[wv, bv] + rank
        idx_slots[c, slot] = idxb[m][order]
        off_slots[c, slot] = off[m][order]
        val_slots[c, slot] = vals[m][order]

    return dict(
        n_core=n_core, nwin=nwin, ngrp=ngrp, nbank=nbank,
        C_wb=C_wb, stream=stream, gathers=gathers,
        total_chunks=total_chunks, total_slots=total_slots,
        idx_slots=idx_slots, off_slots=off_slots, val_slots=val_slots,
    )


def _build_nc(plan, cfg):
    import concourse.bacc as bacc
    import concourse.mybir as mybir
    from concourse import tile
    from concourse import library_config

    N, F, WIN, GRP, BANK = cfg["N"], cfg["F"], cfg["WIN"], cfg["GRP"], cfg["BANK"]
    nwin, ngrp, nbank = plan["nwin"], plan["ngrp"], plan["nbank"]
    n_core = plan["n_core"]
    total_chunks, total_slots = plan["total_chunks"], plan["total_slots"]
    C_wb = plan["C_wb"]
    stream = plan["stream"]
    gathers = plan["gathers"]
    n_core_pad = nwin * WIN
    f16, f32, i16 = mybir.dt.float16, mybir.dt.float32, mybir.dt.int16
    EQ, MUL, MAX = mybir.AluOpType.is_equal, mybir.AluOpType.mult, mybir.AluOpType.max

    nc = bacc.Bacc(None, target_bir_lowering=False, num_swdge_queues=4)
    x16 = nc.dram_tensor("x16", [N, F], f16, kind="ExternalInput")
    idxs = nc.dram_tensor("idxs", [128, max(total_slots // 16, 1)], i16, kind="ExternalInput")
    offs = nc.dram_tensor("offs", [128, max(total_chunks, 1)], f32, kind="ExternalInput")
    valt = nc.dram_tensor("valt", [128, max(total_chunks, 1)], f32, kind="ExternalInput")
    maskc = nc.dram_tensor("maskc", [n_core_pad, F], f32, kind="ExternalInput")
    degwc = nc.dram_tensor("degwc", [1, n_core_pad], f32, kind="ExternalInput")
    wmat = nc.dram_tensor("wmat", [F, F], f32, kind="ExternalInput")
    bvec = nc.dram_tensor("bvec", [1, F], f32, kind="ExternalInput")
    iota = nc.dram_tensor("iota", [128, WIN], f16, kind="ExternalInput")
    out = nc.dram_tensor("out", [n_core, F], f32, kind="ExternalOutput")

    mask_v = maskc.rearrange("(w q p) f -> w p q f", p=128, q=WIN // 128)
    nfull = (n_core // WIN)  # windows fully inside n_core
    if nfull > 0:
        out_v = out[0 : nfull * WIN, :].rearrange("(w q p) f -> w p q f", p=128, q=WIN // 128)

    with tile.TileContext(nc) as tc:
        with tc.tile_pool(name="consts", bufs=1) as consts, \
             tc.tile_pool(name="msgs", bufs=8) as msgs, \
             tc.tile_pool(name="pp", bufs=6) as pp, \
             tc.tile_pool(name="stsb", bufs=3) as stsb, \
             tc.tile_pool(name="outsb", bufs=3) as outsb, \
             tc.tile_pool(name="masksb", bufs=3) as masksb, \
             tc.tile_pool(name="stp", bufs=GRP + 1, space="PSUM") as stp, \
             tc.tile_pool(name="outp", bufs=2, space="PSUM") as outp:

            nc.gpsimd.load_library(library_config.mlp)

            idxt = consts.tile([128, max(total_slots // 16, 1)], i16)
            offt = consts.tile([128, max(total_chunks, 1)], f32)
            valtt = consts.tile([128, max(total_chunks, 1)], f32)
            iotat = consts.tile([128, WIN], f16)
            wt = consts.tile([F, F], f32)
            bt = consts.tile([1, F], f32)
            degwt = consts.tile([1, n_core_pad], f32)
            nc.sync.dma_start(out=idxt[:], in_=idxs[:])
            nc.sync.dma_start(out=offt[:], in_=offs[:])
            nc.sync.dma_start(out=valtt[:], in_=valt[:])
            nc.sync.dma_start(out=iotat[:], in_=iota[:])
            nc.sync.dma_start(out=wt[:], in_=wmat[:])
            nc.sync.dma_start(out=bt[:], in_=bvec[:])
            nc.sync.dma_start(out=degwt[:], in_=degwc[:])

            # chunk -> msgs tile slice, filled in per group
            chunk_src = {}
            st_tiles = {}
            chunks_per_w = C_wb.sum(axis=1)

            # first/last chunk (in stream order) of each window
            first_chunk = {}
            last_chunk = {}
            for (gi, bi, wi, nch, st) in stream:
                for k in range(nch):
                    ci = st + k
                    if wi not in first_chunk:
                        first_chunk[wi] = ci
                    last_chunk[wi] = ci

            # owning group of each chunk (for gather/group association)
            chunk_group = {}
            for (g2, b2, w2, nch2, st2) in stream:
                for k in range(nch2):
                    chunk_group[st2 + k] = g2

            qrr = 0
            for gi in range(ngrp):
                glo, ghi = gi * GRP, min((gi + 1) * GRP, nwin)
                group_gathers = [t for t in gathers if chunk_group[t[1]] == gi]
                for (bi, c0, nch) in group_gathers:
                    mt = msgs.tile([128, 8, F], f16, tag="mt")
                    blo = bi * BANK
                    bhi = min((bi + 1) * BANK, N)
                    nc.gpsimd.dma_gather(
                        mt[:, :nch, :], x16[blo:bhi, :],
                        idxt[:, c0 * 8 : (c0 + nch) * 8],
                        nch * 128, nch * 128, F,
                        queue_num=qrr % 4, single_packet=True,
                    )
                    qrr += 1
                    for k in range(nch):
                        chunk_src[c0 + k] = (mt, k)

                # compute for this group's chunks (stream order)
                for (g2, bi, wi, nch, st) in stream:
                    if g2 != gi or nch == 0:
                        continue
                    if wi not in st_tiles:
                        st_tiles[wi] = stp.tile([128, WIN], f32, tag="st", name=f"st_w{wi}")
                    stt = st_tiles[wi]
                    for k in range(nch):
                        ci = st + k
                        mt, j = chunk_src[ci]
                        P = pp.tile([128, WIN], f16, tag="P")
                        nc.vector.tensor_scalar(
                            P[:], iotat[:], offt[:, ci : ci + 1],
                            valtt[:, ci : ci + 1], EQ, MUL)
                        nc.tensor.matmul(
                            stt[:], mt[:, j, :], P[:],
                            start=(ci == first_chunk[wi]),
                            stop=(ci == last_chunk[wi]))

                # out-stage for this group's windows
                for wi in range(glo, ghi):
                    lo_d = wi * WIN                      # first dest (local)
                    n_d = min(WIN, n_core - lo_d)        # valid dests this window
                    if n_d <= 0:
                        continue
                    nq = -(-n_d // 128)                  # quarters to compute
                    st_sb = stsb.tile([128, WIN], f32, tag="stsb")
                    if chunks_per_w[wi] > 0:
                        nc.scalar.copy(st_sb[:], st_tiles[wi][:])
                    else:
                        nc.vector.memset(st_sb[:], 0.0)
                    op = outp.tile([128, WIN], f32, tag="op")
                    for q in range(nq):
                        nc.tensor.matmul(
                            op[:, q * 128 : (q + 1) * 128],
                            st_sb[:, q * 128 : (q + 1) * 128], wt[:],
                            start=True, stop=False)
                        nc.tensor.matmul(
                            op[:, q * 128 : (q + 1) * 128],
                            degwt[0:1, lo_d + q * 128 : lo_d + (q + 1) * 128],
                            bt[0:1, :], start=False, stop=True)
                    mk = masksb.tile([128, WIN // 128, F], f32, tag="mk")
                    nc.sync.dma_start(out=mk[:, :nq, :], in_=mask_v[wi][:, :nq, :])
                    ob = outsb.tile([128, WIN // 128, F], f32, tag="ob")
                    mkf = mk[:].rearrange("p q f -> p (q f)")
                    obf = ob[:].rearrange("p q f -> p (q f)")
                    nc.vector.scalar_tensor_tensor(
                        obf[:, : nq * 128], op[:, : nq * 128], 0.0,
                        mkf[:, : nq * 128], MAX, MUL)
                    if n_d == WIN:
                        nc.sync.dma_start(out=out_v[wi], in_=ob[:])
                    else:
                        # partial last window: write only valid rows
                        for q in range(nq):
                            nrow = min(128, n_d - q * 128)
                            nc.sync.dma_start(
                                out=out[lo_d + q * 128 : lo_d + q * 128 + nrow, :],
                                in_=ob[:nrow, q, :])
    nc.compile()
    return nc


def _dropout_mask(shape, p, cfg):
    """keep/(1-p) mask identical to the reference's jax threefry dropout."""
    import jax
    cpu = jax.devices("cpu")[0]
    with jax.default_device(cpu):
        keep = jax.random.bernoulli(jax.random.key(42), 1.0 - p, shape)
        keep = np.asarray(keep)
    return (keep.astype(np.float32) / np.float32(1.0 - p)).astype(np.float32)


LAST_EXEC_NS = None


def kernel(X, W, b, rows, cols, vals, cfg=None, _trace=False):
    cfg = dict(DEFAULT_CFG, **(cfg or {}))
    N, F, NC = cfg["N"], cfg["F"], cfg["NC"]
    X = np.ascontiguousarray(np.asarray(X, dtype=np.float32))
    W = np.ascontiguousarray(np.asarray(W, dtype=np.float32))
    b = np.asarray(b, dtype=np.float32).reshape(1, F)
    rows = np.asarray(rows)
    cols = np.asarray(cols)
    vals = np.asarray(vals, dtype=np.float32)

    plan = _plan(rows, cols, vals, cfg)
    n_core, nwin = plan["n_core"], plan["nwin"]
    n_core_pad = nwin * cfg["WIN"]

    mask = _dropout_mask((N, F), cfg["DROP_P"], cfg)
    degw = np.bincount(np.asarray(rows).astype(np.int64),
                       weights=vals.astype(np.float64), minlength=N).astype(np.float32)

    x16 = X.astype(np.float16)
    iota = np.broadcast_to(np.arange(cfg["WIN"], dtype=np.float16), (128, cfg["WIN"]))
    iota = np.ascontiguousarray(iota)

    in_maps = []
    for c in range(NC):
        idx_w = plan["idx_slots"][c].reshape(-1, 16).T  # [16, S/16]
        idx_w = np.ascontiguousarray(np.tile(idx_w, (8, 1)))
        off_t = np.ascontiguousarray(plan["off_slots"][c].reshape(-1, 128).T)
        val_t = np.ascontiguousarray(plan["val_slots"][c].reshape(-1, 128).T)
        mask_c = np.zeros((n_core_pad, F), np.float32)
        mask_c[:n_core] = mask[c * n_core : (c + 1) * n_core]
        degw_c = np.zeros((1, n_core_pad), np.float32)
        degw_c[0, :n_core] = degw[c * n_core : (c + 1) * n_core]
        in_maps.append(dict(
            x16=x16, idxs=idx_w, offs=off_t, valt=val_t,
            maskc=mask_c, degwc=degw_c, wmat=W, bvec=b, iota=iota,
        ))

    nc = _build_nc(plan, cfg)
    from concourse.bass_utils import run_bass_kernel_spmd
    res = run_bass_kernel_spmd(nc, in_maps, core_ids=list(range(NC)), trace=_trace)
    global LAST_EXEC_NS
    LAST_EXEC_NS = res.exec_time_ns
    out = np.concatenate([res.results[c]["out"] for c in range(NC)], axis=0)
    return out


if __name__ == "__main__":
    pass
